# revision 19
# baseline (speedup 1.0000x reference)
"""NT-Xent (SimCLR) loss on 8 Trainium2 NeuronCores.

v2 design, data-parallel over rows of the [2B, 2B] similarity matrix:

- Host casts z0/z1 to bf16 and builds per-core zme (own 1024 rows of
  concat(z0,z1)) and zpr (positive-partner rows).  The program is
  rank-agnostic: per-core behavior differs only through input data.
- All input DMAs write VIRGIN (never-reused) SBUF tiles.  Walrus lowers
  InstDMACopy to a DIRECT2D struct with a single sem-wait slot, so any
  DMA into a reused buffer (WAW+WAR = 2 waits) fails codegen.  20 chunk
  loads of [512,512] bf16 -> [128,2048] tiles keep every DMA at 0 waits.
  A chunk tile holds rows 4p..4p+3 on partition p (sub-tile m = rows
  4p+m), a harmless permutation handled at unshard time.
- The diagonal (self-similarity) is removed algebraically:
  S = sum_j exp(sim_ij/tau) - exp(qd_i/tau), where qd is computed by a
  PE self-matmul Gram diagonal over the same meT tiles used in phase B,
  so qd_i == sim_ii bit-exactly and the exp terms cancel to rounding.
- Positives are rowwise dot(me_n, pr_n) via DVE TTR.
- Host gathers per-row S / positives and does log + mean in float64.
"""

import sys
from contextlib import ExitStack

import numpy as np
import ml_dtypes

for _p in ("/opt/trn_rl_repo", "/opt/pypackages"):
    if _p not in sys.path:
        sys.path.append(_p)

import concourse.bass as bass
import concourse.tile as tile
from concourse import masks, mybir
from concourse.bass_utils import run_bass_kernel_spmd

TAU = 0.07
B = 4096
D = 512
N = 2 * B
NCORES = 8
RPC = N // NCORES        # rows per core = 1024
MT = RPC // 128          # me sub-tiles = 8
NT = N // 512            # sim column blocks = 16
KT = D // 128            # contraction chunks = 4
CH = 512                 # rows per chunk DMA
MEC = RPC // CH          # me/pr chunks = 2
ZC = B // CH             # chunks per z tensor = 8

def _clear_sems_no_rangeclear(self, sems):
    """Replacement for Bass.clear_and_free_semaphores.

    This container's neuronxcc rejects opcode 176
    EVENT_SEMAPHORE_RANGE_CLEAR (visitInstISA "ISA wrong length"), so emit
    one InstEventSemaphore per sem carrying a sem-wr-imm 0 update instead
    (the legacy barrier reset form walrus supports).  Runs between the
    all-engine barriers tile.py places around the clear, so no waits are
    needed on the reset instructions themselves.
    """
    if not sems:
        return
    SH = bass.SemaphoreHandle
    sem_nums = [s.num if isinstance(s, SH) else s for s in sems]
    for rng in bass.compact_to_ranges(sem_nums):
        assert self._state.free_isdisjoint(rng)
        self.gpsimd.dma_reset(rng)
        for s in rng:
            h = self.lookup_sem(s)
            upd = mybir.SyncUpdate(
                sync_type="semaphore", id=s,
                ant_name=h if isinstance(h, str) else h.name,
                update_mode="sem-wr-imm", update_value=0)
            inst = mybir.InstEventSemaphore(
                name=f"semreset_{s}_{self.next_id()}",
                opcode="EventSemaphore",
                engine=mybir.EngineType.Pool,
                ins=[], outs=[],
                sync_info=mybir.SyncInfo(on_wait=[], on_update=[upd]),
                bass_nofuse=True)
            self.gpsimd.add_instruction(inst)
    self._state.prepend_free_semaphores(sem_nums)
    for poison_set in self._tile_sem_poison_stack:
        poison_set.update(sem_nums)


bass.Bass.clear_and_free_semaphores = _clear_sems_no_rangeclear


def _legalize_single_wait(nc):
    """Walrus gives every data instruction exactly ONE sem-wait slot.

    Hoist surplus waits onto wait-only EventSemaphore instructions inserted
    just before the owner on the same engine (engines are in-order, so a
    preceding wait is equivalent to a wait on the instruction itself).
    """
    n = 0
    for fn in nc.m.functions:
        for blk in fn.blocks:
            out = []
            for inst in blk.instructions:
                si = inst.sync_info
                if si is not None and len(si.on_wait) > 1:
                    for w in si.on_wait[:-1]:
                        n += 1
                        out.append(mybir.InstEventSemaphore(
                            name=f"waithoist_{n}",
                            opcode="EventSemaphore",
                            engine=inst.engine,
                            ins=[], outs=[],
                            sync_info=mybir.SyncInfo(
                                on_wait=[w], on_update=[]),
                            bass_nofuse=True))
                    inst.sync_info = mybir.SyncInfo(
                        on_wait=[si.on_wait[-1]], on_update=list(si.on_update))
                out.append(inst)
            blk.instructions[:] = out
    return n

F32 = mybir.dt.float32
BF16 = mybir.dt.bfloat16
AF = mybir.ActivationFunctionType
ALU = mybir.AluOpType
AX = mybir.AxisListType
BF = ml_dtypes.bfloat16


def build_program():
    nc = bass.Bass("TRN2")
    z0b = nc.dram_tensor("z0b", [B, D], BF16, kind="ExternalInput")
    z1b = nc.dram_tensor("z1b", [B, D], BF16, kind="ExternalInput")
    zmeb = nc.dram_tensor("zmeb", [RPC, D], BF16, kind="ExternalInput")
    zprb = nc.dram_tensor("zprb", [RPC, D], BF16, kind="ExternalInput")
    # out[:, 0:MT]    = per-row sum_j exp(sim/tau), diagonal removed
    # out[:, MT:2*MT] = per-row positive similarity
    out = nc.dram_tensor("out", [128, 2 * MT], F32, kind="ExternalOutput")

    with tile.TileContext(nc) as tc, ExitStack() as ctx:
        # virgin chunk tiles: each DMA target is written exactly once
        chup = ctx.enter_context(tc.tile_pool(name="chunks", bufs=1))
        mech = [chup.tile([128, 4 * D], BF16, name=f"mech{i}") for i in range(MEC)]
        prch = [chup.tile([128, 4 * D], BF16, name=f"prch{i}") for i in range(MEC)]
        z0ch = [chup.tile([128, 4 * D], BF16, name=f"z0ch{i}") for i in range(ZC)]
        z1ch = [chup.tile([128, 4 * D], BF16, name=f"z1ch{i}") for i in range(ZC)]
        for i in range(MEC):
            nc.sync.dma_start(out=mech[i][:], in_=zmeb[i * CH:(i + 1) * CH, :])
            nc.sync.dma_start(out=prch[i][:], in_=zprb[i * CH:(i + 1) * CH, :])
        for i in range(ZC):
            nc.sync.dma_start(out=z0ch[i][:], in_=z0b[i * CH:(i + 1) * CH, :])
        for i in range(ZC):
            nc.sync.dma_start(out=z1ch[i][:], in_=z1b[i * CH:(i + 1) * CH, :])

        singles = ctx.enter_context(tc.tile_pool(name="singles", bufs=1))
        ident = singles.tile([128, 128], BF16)
        masks.make_identity(nc, ident[:])
        identf = singles.tile([128, 128], F32)
        masks.make_identity(nc, identf[:])

        tabp = ctx.enter_context(tc.tile_pool(name="tab", bufs=1))
        # tableK[k][d, t] = normalized bf16 embedding, transposed
        tableK = [tabp.tile([128, N], BF16, name=f"tableK{k}") for k in range(KT)]
        meT = [tabp.tile([128, RPC], BF16, name=f"meT{k}") for k in range(KT)]

        sq = ctx.enter_context(tc.tile_pool(name="sq", bufs=2))
        nrm = ctx.enter_context(tc.tile_pool(name="nrm", bufs=8))
        zn = ctx.enter_context(tc.tile_pool(name="zn", bufs=4))
        dgp = ctx.enter_context(tc.tile_pool(name="dg", bufs=2))
        pst = ctx.enter_context(tc.tile_pool(name="pst", bufs=2, space="PSUM"))
        psq = ctx.enter_context(tc.tile_pool(name="psq", bufs=2, space="PSUM"))
        psim = ctx.enter_context(tc.tile_pool(name="psim", bufs=4, space="PSUM"))
        expsc = ctx.enter_context(tc.tile_pool(name="expsc", bufs=4))

        def normalize(src_ap):
            """Row-normalize a [128, D] bf16 slice -> bf16 tile.

            Identical op shapes everywhere so the same input row always
            produces bit-identical normalized output (needed for the
            algebraic diagonal cancellation).
            """
            sqt = sq.tile([128, D], F32)
            ssq = nrm.tile([128, 1], F32)
            nc.scalar.activation(sqt[:], src_ap, AF.Square, accum_out=ssq[:])
            nv = nrm.tile([128, 1], F32)
            nc.scalar.sqrt(nv[:], ssq[:])
            r = nrm.tile([128, 1], F32)
            nc.vector.reciprocal(r[:], nv[:])
            znt = zn.tile([128, D], BF16)
            nc.scalar.mul(znt[:], src_ap, r[:])
            return znt

        def transpose_into(znt, col0, dests):
            for k in range(KT):
                pt = pst.tile([128, 128], BF16)
                nc.tensor.transpose(pt[:], znt[:, k * 128:(k + 1) * 128], ident[:])
                nc.vector.tensor_copy(dests[k][:, col0:col0 + 128], pt[:])

        qpt = singles.tile([128, MT], F32)
        qdt = singles.tile([128, MT], F32)

        # --- phase A-me: own + partner rows -> meT, positives ---
        for ch in range(MEC):
            for m in range(4):
                mt = ch * 4 + m
                a = normalize(mech[ch][:, m * D:(m + 1) * D])
                transpose_into(a, mt * 128, meT)
                b = normalize(prch[ch][:, m * D:(m + 1) * D])
                # TTR is ISA-encoded (opcode 180) and this walrus rejects it
                # ("ISA wrong length"), so use native TensorTensor + reduce.
                s = sq.tile([128, D], F32)
                nc.vector.tensor_mul(s[:], a[:], b[:])
                nc.vector.reduce_sum(qpt[:, mt:mt + 1], s[:], axis=AX.X)

        # warm identf on DVE: TensorTensor has a single walrus wait slot, so
        # the Pool->DVE wait for make_identity must land on this copy, not on
        # the first qd tensor_mul (which already carries the PE wait)
        dwarm = dgp.tile([128, 128], F32)
        nc.vector.tensor_copy(dwarm[:], identf[:])

        # --- qd via PE self-matmul Gram diagonal (bit-exact vs phase B) ---
        for mt in range(MT):
            ps = psq.tile([128, 128], F32)
            for k in range(KT):
                blk = meT[k][:, mt * 128:(mt + 1) * 128]
                nc.tensor.matmul(ps[:], blk, blk,
                                 start=(k == 0), stop=(k == KT - 1))
            dg = dgp.tile([128, 128], F32)
            nc.vector.tensor_mul(dg[:], ps[:], identf[:])
            nc.vector.reduce_sum(qdt[:, mt:mt + 1], dg[:], axis=AX.X)
        dcorr = singles.tile([128, MT], F32)
        nc.scalar.activation(dcorr[:], qdt[:], AF.Exp, scale=1.0 / TAU)

        # --- phase A-table: full normalized transposed table ---
        for ci, chunks in ((0, z0ch), (ZC, z1ch)):
            for i, chv in enumerate(chunks):
                for m in range(4):
                    tt = (ci + i) * 4 + m
                    a = normalize(chv[:, m * D:(m + 1) * D])
                    transpose_into(a, tt * 128, tableK)

        # --- phase B: sim rows, exp, row-sum ---
        accs = singles.tile([128, MT * NT], F32)
        for mt in range(MT):
            for nt in range(NT):
                ps = psim.tile([128, 512], F32)
                for k in range(KT):
                    nc.tensor.matmul(
                        ps[:], meT[k][:, mt * 128:(mt + 1) * 128],
                        tableK[k][:, nt * 512:(nt + 1) * 512],
                        start=(k == 0), stop=(k == KT - 1))
                esc = expsc.tile([128, 512], F32)
                nc.scalar.activation(
                    esc[:], ps[:], AF.Exp, scale=1.0 / TAU,
                    accum_out=accs[:, mt * NT + nt:mt * NT + nt + 1])

        # --- finalize: S = sum(exp) - exp(qd/tau); emit [S | pos] ---
        outS = singles.tile([128, 2 * MT], F32)
        for mt in range(MT):
            red = nrm.tile([128, 1], F32)
            nc.vector.reduce_sum(red[:], accs[:, mt * NT:(mt + 1) * NT], axis=AX.X)
            nc.vector.tensor_sub(outS[:, mt:mt + 1], red[:], dcorr[:, mt:mt + 1])
        nc.vector.tensor_copy(outS[:, MT:2 * MT], qpt[:])
        # SWDGE: its own queue, so the only wait is the DVE writer of outS
        # (HWDGE would add a same-queue ordering wait -> 2 waits -> walrus
        # DIRECT2D single-slot failure)
        nc.gpsimd.dma_start(out=out[:, :], in_=outS[:])

    _legalize_single_wait(nc)
    return nc


_PROGRAM = None


def _get_program():
    global _PROGRAM
    if _PROGRAM is None:
        _PROGRAM = build_program()
    return _PROGRAM


_IDX = None


def _perm_idx():
    """_perm_idx()[mt, p] = core-local row index held at out[p, mt]."""
    global _IDX
    if _IDX is None:
        idx = np.empty((MT, 128), np.int64)
        for mt in range(MT):
            ch, m = divmod(mt, 4)
            idx[mt] = ch * CH + 4 * np.arange(128) + m
        _IDX = idx
    return _IDX


def _run(z0, z1):
    z0b = np.ascontiguousarray(np.asarray(z0, np.float32).astype(BF))
    z1b = np.ascontiguousarray(np.asarray(z1, np.float32).astype(BF))
    zc = np.concatenate([z0b, z1b], axis=0)
    zp = np.concatenate([z1b, z0b], axis=0)  # partner of row g is (g+B) mod 2B
    in_maps = [
        {
            "z0b": z0b,
            "z1b": z1b,
            "zmeb": np.ascontiguousarray(zc[c * RPC:(c + 1) * RPC]),
            "zprb": np.ascontiguousarray(zp[c * RPC:(c + 1) * RPC]),
        }
        for c in range(NCORES)
    ]
    res = run_bass_kernel_spmd(
        _get_program(), in_maps, list(range(NCORES)), trace=False)
    S = np.empty((N,), np.float64)
    pos = np.empty((N,), np.float64)
    idx = _perm_idx()
    for c in range(NCORES):
        o = np.asarray(res.results[c]["out"], np.float64)
        for mt in range(MT):
            S[c * RPC + idx[mt]] = o[:, mt]
            pos[c * RPC + idx[mt]] = o[:, MT + mt]
    loss = (np.log(S).sum() - pos.sum() / TAU) / N
    return np.float32(loss), res


def kernel(z0, z1):
    loss, _ = _run(z0, z1)
    return loss
